# revision 25
# baseline (speedup 1.0000x reference)
"""Trainium2 Bass kernel for nn_MemoryAsContextTransformer.

Sharding: pure data-parallel over the flattened (B*S)=8192 token axis.
Each of the 8 cores handles 1024 contiguous tokens = 2 attention segments
(SEG=512), so the block-diagonal attention never crosses a core boundary
and no collectives are needed.

On-chip layout: activations are kept feature-major ([dim partitions, token
free]) so the whole linear chain (qkv -> attention -> out-proj -> GEGLU FF
-> logits) runs without transposes; per-token scalars (rms-norm, softmax
denominators) are broadcast across partitions with tiny K=1 matmuls.
Attention softmax is computed in [key, query] layout without max
subtraction (logits are O(0.3) here), with causal masking done by zeroing
exp() outputs below the block diagonal.
"""

import numpy as np
import ml_dtypes

# ---- model dims (hardcoded per problem spec) ----
DEPTH = 2
DIM = 512
HEADS = 8
DH = 64
SEG = 512
PM = 4
VOCAB = 32000
B = 2
S = 4096
HD = HEADS * DH  # 512
FFI = 1365  # GEGLU inner
NCORES = 8
NTOK = B * S // NCORES  # 1024 tokens per core
TT = NTOK // 128  # 8 token tiles
DC = DIM // 128  # 4 dim chunks
NSEG = NTOK // SEG  # 2 segments per core
VCH = 500  # vocab chunk
NVC = VOCAB // VCH  # 64
# FF blocks: (a-row offset, rows)
FB = [(i * 128, min(128, FFI - i * 128)) for i in range(11)]
EPS = 1e-6

_cache = {}


def _build_program():
    import concourse.bass as bass
    import concourse.mybir as mybir
    import concourse.tile as tile
    from concourse import bacc
    from concourse.masks import make_identity

    dt = mybir.dt
    f32, bf16, i32 = dt.float32, dt.float16, dt.int32
    AF = mybir.ActivationFunctionType

    nc = bacc.Bacc("TRN2", target_bir_lowering=False, debug=False)

    def din(name, shape, dtype):
        return nc.dram_tensor(name, shape, dtype, kind="ExternalInput")

    tokidx = din("tokidx", [TT, 128, 1], i32)
    possl = din("possl", [NTOK, DIM], f32)
    tokemb = din("tokemb", [VOCAB, DIM], f32)
    cosb = din("cosb", [128, NTOK], bf16)
    sinb = din("sinb", [128, NTOK], bf16)
    rmat = din("rmat", [128, 128], bf16)
    trimask = din("trimask", [128, 128], bf16)
    wqk = din("wqk", [DEPTH, DIM, 1024], bf16)
    wvm = din("wvm", [DEPTH, DIM, 520], bf16)
    vmixb = din("vmixb", [128, HEADS], f32)
    pmk = din("pmk", [DEPTH, 128, 4, PM], bf16)  # [.., head-pair, pm] lhsT
    pmv = din("pmv", [DEPTH, PM, HEADS, DH + 1], bf16)  # with ones col
    woutw = din("woutw", [DEPTH, HD, DIM], bf16)
    w1 = din("w1", [DEPTH, DIM, 2 * FFI], bf16)  # a/g interleaved blocks
    b1 = din("b1", [DEPTH, 2 * FFI, 1], f32)  # permuted to match w1
    w2 = din("w2", [DEPTH, FFI, DIM], bf16)
    b2 = din("b2", [DEPTH, 128, DC], f32)
    # wl pre-swizzled host-side: [vc, p, dc*500+j] = wl_eff[dc*128+p, vc*500+j]
    # so each partition's line per vocab chunk is 4KB contiguous in DRAM.
    wl = din("wl", [NVC, 128, DC * VCH], bf16)
    out = nc.dram_tensor("out", [NTOK, VOCAB], f32, kind="ExternalOutput")

    with tile.TileContext(nc) as tc:
        # ---------- persistent pools ----------
        const = tc.alloc_tile_pool(name="const", bufs=1)
        persist = tc.alloc_tile_pool(name="persist", bufs=1)

        ident = const.tile([128, 128], f32)
        make_identity(nc, ident[:])
        tri_sb = const.tile([128, 128], bf16)
        nc.sync.dma_start(tri_sb[:], trimask[:])
        rmat_sb = const.tile([128, 128], bf16)
        nc.sync.dma_start(rmat_sb[:], rmat[:])
        cos_sb = const.tile([128, NTOK], bf16)
        nc.sync.dma_start(cos_sb[:], cosb[:])
        sin_sb = const.tile([128, NTOK], bf16)
        nc.sync.dma_start(sin_sb[:], sinb[:])
        ones_bf = const.tile([128, 128], bf16)
        nc.vector.memset(ones_bf[:], 1.0)
        ones_f32 = const.tile([128, 64], f32)
        nc.vector.memset(ones_f32[:], 1.0)
        eps_sb = const.tile([128, 1], f32)
        nc.vector.memset(eps_sb[:], EPS)
        vb_sb = const.tile([128, HEADS], f32)
        nc.sync.dma_start(vb_sb[:], vmixb[:])

        x_fm = persist.tile([128, DC, NTOK], f32)  # residual stream, fm
        v0_tm = persist.tile([128, TT, HEADS, DH + 1], bf16)  # layer-0 v
        v1_tm = persist.tile([128, TT, HEADS, DH + 1], bf16)
        qk_bf = persist.tile([128, 8, NTOK], bf16)  # q|k pre-rope
        qkr_bf = persist.tile([128, 8, NTOK], bf16)  # q|k post-rope
        o_asm = persist.tile([128, DC, NTOK], bf16)  # attn out, fm
        h_sb = persist.tile([128, 11, NTOK], bf16)  # GEGLU hidden
        xn_bf = persist.tile([128, DC, NTOK], bf16)  # normed activations

        # ---------- embedding: gather + pos, transpose to fm ----------
        with (
            tc.tile_pool(name="emb", bufs=3) as gpool,
            tc.tile_pool(name="embi", bufs=3) as ipool,
            tc.tile_pool(name="embp", bufs=3, space="PSUM") as tr_ps,
        ):
            for t in range(TT):
                idx_sb = ipool.tile([128, 1], i32)
                nc.sync.dma_start(idx_sb[:], tokidx[t])
                g_sb = gpool.tile([128, DIM], f32, tag="g")
                nc.gpsimd.indirect_dma_start(
                    out=g_sb[:],
                    out_offset=None,
                    in_=tokemb[:],
                    in_offset=bass.IndirectOffsetOnAxis(ap=idx_sb[:, :1], axis=0),
                )
                p_sb = gpool.tile([128, DIM], f32, tag="p")
                nc.sync.dma_start(p_sb[:], possl[t * 128 : (t + 1) * 128, :])
                nc.vector.tensor_add(g_sb[:], g_sb[:], p_sb[:])
                for c in range(DC):
                    tp = tr_ps.tile([128, 128], f32)
                    nc.tensor.transpose(tp[:], g_sb[:, c * 128 : (c + 1) * 128], ident[:])
                    nc.vector.tensor_copy(x_fm[:, c, t * 128 : (t + 1) * 128], tp[:])

        # ---------- helpers ----------
        def rmsnorm_to(dst_bf, ln_pool, ln_ps):
            """dst[:, dc, :] = x_fm * invrms (weights folded into W), fp16."""
            xsq = ln_pool.tile([128, DC, NTOK], bf16, tag="xsq")
            for c in range(DC):
                nc.vector.tensor_mul(xsq[:, c, :], x_fm[:, c, :], x_fm[:, c, :])
            for half in range(2):
                cols = slice(half * 512, half * 512 + 512)
                ssq = ln_ps.tile([1, 512], f32, tag="ssq")
                for c in range(DC):
                    nc.tensor.matmul(
                        ssq[:], ones_bf[:, 0:1], xsq[:, c, cols],
                        start=(c == 0), stop=(c == DC - 1),
                    )
                inv = ln_pool.tile([128, 512], bf16, tag="inv")
                rtmp = ln_pool.tile([128, 512], f32, tag="rtmp")
                nc.scalar.activation(
                    rtmp[0:1, :], ssq[:], AF.Sqrt, bias=eps_sb[0:1], scale=1.0 / DIM
                )
                with nc.allow_low_precision(reason="fp16 invrms feeds fp16 matmul"):
                    nc.vector.reciprocal(inv[0:1, :], rtmp[0:1, :])
                bc = ln_ps.tile([128, 512], f32, tag="bc")
                nc.tensor.matmul(bc[:], ones_bf[0:1, :], inv[0:1, :], start=True, stop=True)
                for c in range(DC):
                    nc.vector.tensor_mul(dst_bf[:, c, cols], x_fm[:, c, cols], bc[:])

        # ---------- layers ----------
        for d in range(DEPTH):
            with (
                tc.tile_pool(name=f"ln{d}", bufs=2) as ln_pool,
                tc.tile_pool(name=f"lnp{d}", bufs=2, space="PSUM") as ln_ps,
            ):
                rmsnorm_to(xn_bf, ln_pool, ln_ps)

            # -- qk projection (feature-major) --
            with (
                tc.tile_pool(name=f"wqk{d}", bufs=1) as wpool,
                tc.tile_pool(name=f"qkp{d}", bufs=4, space="PSUM") as qk_ps,
            ):
                w_sb = wpool.tile([128, DC, 1024], bf16)
                nc.sync.dma_start(
                    w_sb[:], wqk[d].rearrange("(dc p) f -> p dc f", p=128)
                )
                for fc in range(8):
                    for half in range(2):
                        cols = slice(half * 512, half * 512 + 512)
                        ps = qk_ps.tile([128, 512], f32)
                        for c in range(DC):
                            nc.tensor.matmul(
                                ps[:],
                                w_sb[:, c, fc * 128 : (fc + 1) * 128],
                                xn_bf[:, c, cols],
                                start=(c == 0), stop=(c == DC - 1),
                            )
                        nc.scalar.copy(qk_bf[:, fc, cols], ps[:])

            # -- rope (feature-major, rotation via PE) --
            with (
                tc.tile_pool(name=f"rp{d}", bufs=4) as rpool,
                tc.tile_pool(name=f"rpp{d}", bufs=3, space="PSUM") as r_ps,
            ):
                for fc in range(8):
                    for half in range(2):
                        cols = slice(half * 512, half * 512 + 512)
                        rot = r_ps.tile([128, 512], f32)
                        nc.tensor.matmul(
                            rot[:], rmat_sb[:], qk_bf[:, fc, cols], start=True, stop=True
                        )
                        rot_sb = rpool.tile([128, 512], bf16, tag="rot")
                        nc.vector.tensor_copy(rot_sb[:], rot[:])
                        t1 = rpool.tile([128, 512], bf16, tag="t1")
                        nc.vector.tensor_mul(t1[:], qk_bf[:, fc, cols], cos_sb[:, cols])
                        nc.vector.tensor_mul(rot_sb[:], rot_sb[:], sin_sb[:, cols])
                        nc.vector.tensor_add(qkr_bf[:, fc, cols], t1[:], rot_sb[:])

            # -- v + mix projection (token-major / key-major) --
            v_tm = v0_tm if d == 0 else v1_tm
            with (
                tc.tile_pool(name=f"wv{d}", bufs=1) as wvpool,
                tc.tile_pool(name=f"vp{d}", bufs=3, space="PSUM") as v_ps,
                tc.tile_pool(name=f"vm{d}", bufs=3) as vtmp,
            ):
                wv_sb = wvpool.tile([128, DC, 520], bf16)
                nc.sync.dma_start(
                    wv_sb[:], wvm[d].rearrange("(dc p) f -> p dc f", p=128)
                )
                for t in range(TT):
                    trow = slice(t * 128, (t + 1) * 128)
                    ps = v_ps.tile([128, 512], f32, tag="v")
                    for c in range(DC):
                        nc.tensor.matmul(
                            ps[:], xn_bf[:, c, trow], wv_sb[:, c, 0:512],
                            start=(c == 0), stop=(c == DC - 1),
                        )
                    if d == 0:
                        nc.scalar.copy(
                            v_tm[:, t, :, 0:DH],
                            ps[:].rearrange("p (h e) -> p h e", h=HEADS),
                        )
                    else:
                        ps8 = v_ps.tile([128, 8], f32, tag="m")
                        for c in range(DC):
                            nc.tensor.matmul(
                                ps8[:], xn_bf[:, c, trow], wv_sb[:, c, 512:520],
                                start=(c == 0), stop=(c == DC - 1),
                            )
                        mixf = vtmp.tile([128, 8], f32, tag="mixf")
                        nc.vector.tensor_add(mixf[:], ps8[:], vb_sb[:])
                        mix = vtmp.tile([128, 8], bf16, tag="mix")
                        nc.scalar.activation(mix[:], mixf[:], AF.Sigmoid)
                        v1 = vtmp.tile([128, HEADS, DH], bf16, tag="v1")
                        nc.scalar.copy(v1[:], ps[:].rearrange("p (h e) -> p h e", h=HEADS))
                        dv = vtmp.tile([128, HEADS, DH], bf16, tag="dv")
                        nc.vector.tensor_sub(dv[:], v0_tm[:, t, :, 0:DH], v1[:])
                        nc.vector.tensor_mul(
                            dv[:], dv[:], mix[:, :, None].to_broadcast([128, HEADS, DH])
                        )
                        nc.vector.tensor_add(v_tm[:, t, :, 0:DH], v1[:], dv[:])
                nc.vector.memset(v_tm[:, :, :, DH : DH + 1], 1.0)

            # -- attention --
            with (
                tc.tile_pool(name=f"pm{d}", bufs=1) as pmpool,
                tc.tile_pool(name=f"ep{d}", bufs=6) as epool,
                tc.tile_pool(name=f"at{d}", bufs=4) as apool,
                tc.tile_pool(name=f"simp{d}", bufs=2, space="PSUM") as sim_ps,
                tc.tile_pool(name=f"op{d}", bufs=2, space="PSUM") as o_ps_pool,
                tc.tile_pool(name=f"bcp{d}", bufs=2, space="PSUM") as bc_ps_pool,
            ):
                pmk_sb = pmpool.tile([128, 4, PM], bf16)
                nc.sync.dma_start(pmk_sb[:], pmk[d])
                pmv_sb = pmpool.tile([PM, HEADS, DH + 1], bf16)
                nc.sync.dma_start(pmv_sb[:], pmv[d])
                for s in range(NSEG):
                    scols = slice(s * 512, (s + 1) * 512)
                    for h in range(HEADS):
                        base = (h % 2) * 64
                        fcq = h // 2
                        fck = 4 + h // 2
                        q_ap = qkr_bf[base : base + 64, fcq, scols]
                        e_tiles = []
                        for c in range(4):
                            sp = sim_ps.tile([128, 512], f32, tag="sim")
                            k_ap = qkr_bf[
                                base : base + 64, fck,
                                s * 512 + c * 128 : s * 512 + (c + 1) * 128,
                            ]
                            nc.tensor.matmul(sp[:], k_ap, q_ap, start=True, stop=True)
                            e_c = epool.tile([128, 512], bf16, tag="e")
                            nc.scalar.activation(
                                e_c[:, c * 128 :], sp[:, c * 128 :], AF.Exp,
                                scale=DH**-0.5,
                            )
                            if c > 0:
                                nc.gpsimd.memset(e_c[:, : c * 128], 0.0)
                            nc.vector.tensor_mul(
                                e_c[:, c * 128 : (c + 1) * 128],
                                e_c[:, c * 128 : (c + 1) * 128],
                                tri_sb[:],
                            )
                            e_tiles.append(e_c)
                        pp = sim_ps.tile([PM, 512], f32, tag="sim")
                        nc.tensor.matmul(
                            pp[:], pmk_sb[base : base + 64, fcq, :], q_ap,
                            start=True, stop=True,
                        )
                        e_pm = epool.tile([PM, 512], bf16, tag="epm")
                        nc.scalar.activation(e_pm[:], pp[:], AF.Exp, scale=DH**-0.5)
                        # o (rows 0..63) + denom (row 64)
                        op = o_ps_pool.tile([DH + 1, 512], f32)
                        for c in range(4):
                            nc.tensor.matmul(
                                op[:],
                                v_tm[:, 4 * s + c, h, :],
                                e_tiles[c][:],
                                start=(c == 0), stop=False,
                            )
                        nc.tensor.matmul(
                            op[:], pmv_sb[:, h, :], e_pm[:], start=False, stop=True
                        )
                        invd = apool.tile([128, 512], bf16, tag="invd")
                        with nc.allow_low_precision(reason="fp16 inv-denom"):
                            nc.vector.reciprocal(invd[64:65, :], op[64:65, :])
                        bc = bc_ps_pool.tile([64, 512], f32)
                        nc.tensor.matmul(
                            bc[:], ones_bf[64:65, 0:64], invd[64:65, :],
                            start=True, stop=True,
                        )
                        o_f = apool.tile([64, 512], f32, tag="of")
                        nc.scalar.copy(o_f[:], op[0:64, :])
                        if h % 2 == 0:
                            nc.vector.tensor_mul(o_asm[0:64, fcq, scols], o_f[:], bc[:])
                        else:
                            o_tmp = apool.tile([64, 512], bf16, tag="otmp")
                            nc.vector.tensor_mul(o_tmp[:], o_f[:], bc[:])
                            nc.sync.dma_start(o_asm[64:128, fcq, scols], o_tmp[:])

            # -- output projection + residual --
            with (
                tc.tile_pool(name=f"wo{d}", bufs=1) as wopool,
                tc.tile_pool(name=f"wop{d}", bufs=3, space="PSUM") as wo_ps,
            ):
                wo_sb = wopool.tile([128, 4, 512], bf16)
                nc.sync.dma_start(
                    wo_sb[:], woutw[d].rearrange("(kc p) m -> p kc m", p=128)
                )
                for mc in range(DC):
                    for half in range(2):
                        cols = slice(half * 512, half * 512 + 512)
                        ps = wo_ps.tile([128, 512], f32)
                        for kc in range(4):
                            nc.tensor.matmul(
                                ps[:],
                                wo_sb[:, kc, mc * 128 : (mc + 1) * 128],
                                o_asm[:, kc, cols],
                                start=(kc == 0), stop=(kc == 3),
                            )
                        nc.vector.tensor_add(x_fm[:, mc, cols], x_fm[:, mc, cols], ps[:])

            # -- GEGLU FF --
            with (
                tc.tile_pool(name=f"ln2{d}", bufs=2) as ln_pool,
                tc.tile_pool(name=f"ln2p{d}", bufs=2, space="PSUM") as ln_ps,
            ):
                rmsnorm_to(xn_bf, ln_pool, ln_ps)
            with (
                tc.tile_pool(name=f"wf{d}", bufs=1) as wfpool,
                tc.tile_pool(name=f"fb{d}", bufs=1) as fbpool,
                tc.tile_pool(name=f"ffp{d}", bufs=2, space="PSUM") as ff_ps,
                tc.tile_pool(name=f"fft{d}", bufs=4) as ftmp,
            ):
                w1_sb = wfpool.tile([128, DC, 2 * FFI], bf16)
                nc.sync.dma_start(
                    w1_sb[:], w1[d].rearrange("(dc p) f -> p dc f", p=128)
                )
                w2_sb = wfpool.tile([128, 11, 512], bf16)
                nc.sync.dma_start(
                    w2_sb[:, 0:10, :],
                    w2[d, 0:1280, :].rearrange("(kb p) m -> p kb m", p=128),
                )
                nc.sync.dma_start(w2_sb[0:85, 10, :], w2[d, 1280:1365, :])
                b1a = []
                b1g = []
                for i in range(11):
                    pa = FB[i][1]
                    ta = fbpool.tile([128, 1], f32, name=f"b1a{d}_{i}")
                    tg = fbpool.tile([128, 1], f32, name=f"b1g{d}_{i}")
                    nc.sync.dma_start(ta[0:pa, :], b1[d, 256 * i : 256 * i + pa, :])
                    nc.sync.dma_start(
                        tg[0:pa, :], b1[d, 256 * i + pa : 256 * i + 2 * pa, :]
                    )
                    b1a.append(ta)
                    b1g.append(tg)
                b2_sb = fbpool.tile([128, DC], f32)
                nc.sync.dma_start(b2_sb[:], b2[d])
                for i in range(11):
                    pa = FB[i][1]
                    for half in range(2):
                        cols = slice(half * 512, half * 512 + 512)
                        aps = ff_ps.tile([128, 512], f32, tag="a")
                        gps = ff_ps.tile([128, 512], f32, tag="g")
                        for c in range(DC):
                            nc.tensor.matmul(
                                aps[0:pa, :],
                                w1_sb[:, c, 256 * i : 256 * i + pa],
                                xn_bf[:, c, cols],
                                start=(c == 0), stop=(c == DC - 1),
                            )
                        for c in range(DC):
                            nc.tensor.matmul(
                                gps[0:pa, :],
                                w1_sb[:, c, 256 * i + pa : 256 * i + 2 * pa],
                                xn_bf[:, c, cols],
                                start=(c == 0), stop=(c == DC - 1),
                            )
                        g_sb = ftmp.tile([128, 512], bf16, tag="gs")
                        nc.scalar.activation(
                            g_sb[0:pa, :], gps[0:pa, :], AF.Identity, bias=b1g[i][0:pa]
                        )
                        sig_sb = ftmp.tile([128, 512], bf16, tag="sg")
                        nc.scalar.activation(
                            sig_sb[0:pa, :], gps[0:pa, :], AF.Sigmoid, bias=b1g[i][0:pa]
                        )
                        a_sb = ftmp.tile([128, 512], bf16, tag="as")
                        nc.vector.tensor_scalar_add(
                            a_sb[0:pa, :], aps[0:pa, :], b1a[i][0:pa]
                        )
                        nc.vector.tensor_mul(
                            a_sb[0:pa, :], a_sb[0:pa, :], g_sb[0:pa, :]
                        )
                        nc.vector.tensor_mul(
                            h_sb[0:pa, i, cols], a_sb[0:pa, :], sig_sb[0:pa, :]
                        )
                for mc in range(DC):
                    for half in range(2):
                        cols = slice(half * 512, half * 512 + 512)
                        ps = ff_ps.tile([128, 512], f32, tag="o2")
                        for kb in range(11):
                            pa = FB[kb][1]
                            nc.tensor.matmul(
                                ps[:],
                                w2_sb[0:pa, kb, mc * 128 : (mc + 1) * 128],
                                h_sb[0:pa, kb, cols],
                                start=(kb == 0), stop=(kb == 10),
                            )
                        t_sb = ftmp.tile([128, 512], f32, tag="t2")
                        nc.scalar.activation(
                            t_sb[:], ps[:], AF.Identity, bias=b2_sb[:, mc : mc + 1]
                        )
                        nc.vector.tensor_add(x_fm[:, mc, cols], x_fm[:, mc, cols], t_sb[:])

        # ---------- final norm + logits ----------
        with (
            tc.tile_pool(name="lnf", bufs=2) as ln_pool,
            tc.tile_pool(name="lnfp", bufs=2, space="PSUM") as ln_ps,
        ):
            rmsnorm_to(xn_bf, ln_pool, ln_ps)
        with (
            tc.tile_pool(name="wl", bufs=3) as wlpool,
            tc.tile_pool(name="lg", bufs=4, space="PSUM") as lg_ps,
            tc.tile_pool(name="lo", bufs=6) as lopool,
        ):
            for vc in range(NVC):
                vcols = slice(vc * VCH, (vc + 1) * VCH)
                wl_sb = wlpool.tile([128, DC, VCH], bf16)
                nc.sync.dma_start(
                    wl_sb[:], wl[vc].rearrange("p (dc v) -> p dc v", dc=DC)
                )
                for t in range(TT):
                    trow = slice(t * 128, (t + 1) * 128)
                    ps = lg_ps.tile([128, VCH], f32)
                    for c in range(DC):
                        nc.tensor.matmul(
                            ps[:], xn_bf[:, c, trow], wl_sb[:, c, :],
                            start=(c == 0), stop=(c == DC - 1),
                        )
                    o_sb = lopool.tile([128, VCH], f32)
                    if (vc * TT + t) % 2 == 0:
                        nc.scalar.copy(o_sb[:], ps[:])
                    else:
                        nc.vector.tensor_copy(o_sb[:], ps[:])
                    nc.sync.dma_start(out[trow, vcols], o_sb[:])

        persist.release()
        const.release()

    nc.compile()
    return nc


def _host_prep(inputs):
    """Build the shared (weights) and per-core input maps."""
    bf = np.float16
    f = lambda x: np.ascontiguousarray(np.asarray(x, np.float32))
    tokens = np.asarray(inputs["tokens"]).astype(np.int32)
    tok_emb = f(inputs["tok_emb"])
    pos_emb = f(inputs["pos_emb"])
    anw = f(inputs["attn_norm_w"])  # [2,512]
    Wqkv = f(inputs["Wqkv"])  # [2,512,1536]
    persist_mem = f(inputs["persist_mem"])  # [2,2,8,4,64]
    Wout = f(inputs["Wout"])
    vmix_w = f(inputs["vmix_w"])  # [2,512,8]
    vmix_b = f(inputs["vmix_b"])  # [2,8]
    fnw = f(inputs["ff_norm_w"])
    ff_w1 = f(inputs["ff_w1"])  # [2,512,2730]
    ff_b1 = f(inputs["ff_b1"])  # [2,2730]
    ff_w2 = f(inputs["ff_w2"])  # [2,1365,512]
    ff_b2 = f(inputs["ff_b2"])  # [2,512]
    finw = f(inputs["final_norm_w"])  # [512]
    w_logits = f(inputs["w_logits"])  # [512,32000]

    # ---- shared tensors ----
    wqk = np.ascontiguousarray((anw[:, :, None] * Wqkv[:, :, :1024]).astype(bf))
    wvm = np.ascontiguousarray(
        np.concatenate(
            [anw[:, :, None] * Wqkv[:, :, 1024:], anw[:, :, None] * vmix_w], axis=2
        ).astype(bf)
    )
    vmixb = np.broadcast_to(vmix_b[1], (128, HEADS)).astype(np.float32).copy()
    # pmk lhsT: [d, r(128), pair(4), pm] ; r<64 -> head 2*pair dh=r ; r>=64 -> head 2*pair+1
    pmk = np.zeros((DEPTH, 128, 4, PM), np.float32)
    for pair in range(4):
        pmk[:, 0:64, pair, :] = persist_mem[:, 0, 2 * pair].transpose(0, 2, 1)
        pmk[:, 64:128, pair, :] = persist_mem[:, 0, 2 * pair + 1].transpose(0, 2, 1)
    pmk = pmk.astype(bf)
    pmv = np.ones((DEPTH, PM, HEADS, DH + 1), np.float32)
    pmv[:, :, :, 0:DH] = persist_mem[:, 1].transpose(0, 2, 1, 3)
    pmv = pmv.astype(bf)
    woutw = Wout.astype(bf)
    # w1: interleave a/g blocks of 128 (last 85), fold ff norm weight
    w1s = fnw[:, :, None] * ff_w1
    w1 = np.zeros((DEPTH, DIM, 2 * FFI), np.float32)
    b1 = np.zeros((DEPTH, 2 * FFI, 1), np.float32)
    for i, (off, pa) in enumerate(FB):
        w1[:, :, 256 * i : 256 * i + pa] = w1s[:, :, off : off + pa]
        w1[:, :, 256 * i + pa : 256 * i + 2 * pa] = w1s[:, :, FFI + off : FFI + off + pa]
        b1[:, 256 * i : 256 * i + pa, 0] = ff_b1[:, off : off + pa]
        b1[:, 256 * i + pa : 256 * i + 2 * pa, 0] = ff_b1[:, FFI + off : FFI + off + pa]
    w1 = w1.astype(bf)
    w2 = ff_w2.astype(bf)
    b2 = np.ascontiguousarray(
        ff_b2.reshape(DEPTH, DC, 128).transpose(0, 2, 1)
    ).astype(np.float32)
    wl_eff = (finw[:, None] * w_logits).astype(bf)  # [512, 32000]
    # swizzle to [vc, p, dc*VCH+j] so per-partition DMA lines are contiguous
    wl = np.ascontiguousarray(
        wl_eff.reshape(DC, 128, NVC, VCH).transpose(2, 1, 0, 3).reshape(NVC, 128, DC * VCH)
    )
    # rope rotation matrix (lhsT): rot = x[2i] -> out[2i+1], -x[2i+1] -> out[2i]
    rmat = np.zeros((128, 128), np.float32)
    for blk in range(2):
        o = blk * 64
        for i in range(32):
            rmat[o + 2 * i + 1, o + 2 * i] = -1.0
            rmat[o + 2 * i, o + 2 * i + 1] = 1.0
    rmat = rmat.astype(bf)
    tri = np.triu(np.ones((128, 128), np.float32)).astype(bf)  # allowed q>=k

    flat_tokens = tokens.reshape(-1)
    inv = (10000.0 ** (-np.arange(0, DH, 2, dtype=np.float64) / DH)).astype(np.float64)
    row_pair = (np.arange(128) % 64) // 2  # pair index per fm row

    in_maps = []
    for c in range(NCORES):
        pos = (c * NTOK + np.arange(NTOK)) % S
        fr = pos[:, None].astype(np.float64) * inv[None, :]  # [1024, 32]
        cosb = np.cos(fr)[:, row_pair].T.astype(bf)  # [128, 1024]
        sinb = np.sin(fr)[:, row_pair].T.astype(bf)
        in_maps.append(
            {
                "tokidx": np.ascontiguousarray(
                    flat_tokens[c * NTOK : (c + 1) * NTOK].reshape(TT, 128, 1)
                ),
                "possl": np.ascontiguousarray(pos_emb[pos]),
                "tokemb": tok_emb,
                "cosb": np.ascontiguousarray(cosb),
                "sinb": np.ascontiguousarray(sinb),
                "rmat": rmat,
                "trimask": tri,
                "wqk": wqk,
                "wvm": wvm,
                "vmixb": vmixb,
                "pmk": pmk,
                "pmv": pmv,
                "woutw": woutw,
                "w1": w1,
                "b1": b1,
                "w2": w2,
                "b2": b2,
                "wl": wl,
            }
        )
    return in_maps


def kernel(**inputs):
    from concourse.bass_utils import run_bass_kernel_spmd

    if "nc" not in _cache:
        _cache["nc"] = _build_program()
    nc = _cache["nc"]
    in_maps = _host_prep(inputs)
    res = run_bass_kernel_spmd(nc, in_maps, core_ids=list(range(NCORES)))
    outs = [res.results[c]["out"] for c in range(NCORES)]
    return np.concatenate(outs, axis=0).reshape(B, S, VOCAB).astype(np.float32)


# revision 28
# speedup vs baseline: 2.0777x; 2.0777x over previous
"""Trainium2 Bass kernel for nn_MemoryAsContextTransformer.

Sharding: pure data-parallel over the flattened (B*S)=8192 token axis.
Each of the 8 cores handles 1024 contiguous tokens = 2 attention segments
(SEG=512), so the block-diagonal attention never crosses a core boundary
and no collectives are needed.

On-chip layout: activations are kept feature-major ([dim partitions, token
free]) so the whole linear chain (qkv -> attention -> out-proj -> GEGLU FF
-> logits) runs without transposes; per-token scalars (rms-norm, softmax
denominators) are broadcast across partitions with tiny K=1 matmuls.
Attention softmax is computed in [key, query] layout without max
subtraction (logits are O(0.3) here), with causal masking done by zeroing
exp() outputs below the block diagonal.
"""

import numpy as np
import ml_dtypes

# ---- model dims (hardcoded per problem spec) ----
DEPTH = 2
DIM = 512
HEADS = 8
DH = 64
SEG = 512
PM = 4
VOCAB = 32000
B = 2
S = 4096
HD = HEADS * DH  # 512
FFI = 1365  # GEGLU inner
NCORES = 8
NTOK = B * S // NCORES  # 1024 tokens per core
TT = NTOK // 128  # 8 token tiles
DC = DIM // 128  # 4 dim chunks
NSEG = NTOK // SEG  # 2 segments per core
VCH = 500  # vocab chunk
NVC = VOCAB // VCH  # 64
# FF blocks: (a-row offset, rows)
FB = [(i * 128, min(128, FFI - i * 128)) for i in range(11)]
EPS = 1e-6

_cache = {}


def _build_program():
    import os
    import concourse.bass as bass
    import concourse.mybir as mybir
    import concourse.tile as tile
    from concourse import bacc
    from concourse.masks import make_identity

    nvc = int(os.environ.get("KERNEL_NVC", NVC))  # debug knob: partial logits
    ndepth = int(os.environ.get("KERNEL_DEPTH", DEPTH))  # debug knob: layers

    dt = mybir.dt
    f32, bf16, i32 = dt.float32, dt.float16, dt.int32
    AF = mybir.ActivationFunctionType

    nc = bacc.Bacc("TRN2", target_bir_lowering=False, debug=False)

    def din(name, shape, dtype):
        return nc.dram_tensor(name, shape, dtype, kind="ExternalInput")

    tokidx = din("tokidx", [TT, 128, 1], i32)
    possl = din("possl", [NTOK, DIM], f32)
    tokemb = din("tokemb", [VOCAB, DIM], f32)
    cosb = din("cosb", [128, NTOK], bf16)
    sinb = din("sinb", [128, NTOK], bf16)
    rmat = din("rmat", [128, 128], bf16)
    trimask = din("trimask", [128, 128], bf16)
    wqk = din("wqk", [DEPTH, DIM, 1024], bf16)
    wvm = din("wvm", [DEPTH, DIM, 520], bf16)
    vmixb = din("vmixb", [128, HEADS], f32)
    pmk = din("pmk", [DEPTH, 128, 4, PM], bf16)  # [.., head-pair, pm] lhsT
    pmv = din("pmv", [DEPTH, PM, HEADS, DH + 1], bf16)  # with ones col
    woutw = din("woutw", [DEPTH, HD, DIM], bf16)
    w1 = din("w1", [DEPTH, DIM, 2 * FFI], bf16)  # a/g interleaved blocks
    b1 = din("b1", [DEPTH, 2 * FFI, 1], f32)  # permuted to match w1
    w2 = din("w2", [DEPTH, FFI, DIM], bf16)
    b2 = din("b2", [DEPTH, 128, DC], f32)
    # wl pre-swizzled host-side: [vc, p, dc*500+j] = wl_eff[dc*128+p, vc*500+j]
    # so each partition's line per vocab chunk is 4KB contiguous in DRAM.
    wl = din("wl", [NVC, 128, DC * VCH], bf16)
    out = nc.dram_tensor("out", [NTOK, VOCAB], f32, kind="ExternalOutput")

    with tile.TileContext(nc) as tc:
        # ---------- persistent pools ----------
        const = tc.alloc_tile_pool(name="const", bufs=1)
        persist = tc.alloc_tile_pool(name="persist", bufs=1)

        ident = const.tile([128, 128], f32)
        make_identity(nc, ident[:])
        tri_sb = const.tile([128, 128], bf16)
        nc.sync.dma_start(tri_sb[:], trimask[:])
        rmat_sb = const.tile([128, 128], bf16)
        nc.sync.dma_start(rmat_sb[:], rmat[:])
        cos_sb = const.tile([128, NTOK], bf16)
        nc.sync.dma_start(cos_sb[:], cosb[:])
        sin_sb = const.tile([128, NTOK], bf16)
        nc.sync.dma_start(sin_sb[:], sinb[:])
        ones_bf = const.tile([128, 128], bf16)
        nc.vector.memset(ones_bf[:], 1.0)
        ones_f32 = const.tile([128, 64], f32)
        nc.vector.memset(ones_f32[:], 1.0)
        eps_sb = const.tile([128, 1], f32)
        nc.vector.memset(eps_sb[:], EPS)
        vb_sb = const.tile([128, HEADS], f32)
        nc.sync.dma_start(vb_sb[:], vmixb[:])

        x_fm = persist.tile([128, DC, NTOK], f32)  # residual stream, fm
        v0_tm = persist.tile([128, TT, HEADS, DH + 1], bf16)  # layer-0 v
        v1_tm = persist.tile([128, TT, HEADS, DH + 1], bf16)
        qk_bf = persist.tile([128, 8, NTOK], bf16)  # q|k pre-rope
        qkr_bf = persist.tile([128, 8, NTOK], bf16)  # q|k post-rope
        o_asm = persist.tile([128, DC, NTOK], bf16)  # attn out, fm
        h_sb = persist.tile([128, 11, NTOK], bf16)  # GEGLU hidden
        xn_bf = persist.tile([128, DC, NTOK], bf16)  # normed activations

        # ---------- embedding: gather + pos, transpose to fm ----------
        with (
            tc.tile_pool(name="emb", bufs=3) as gpool,
            tc.tile_pool(name="embi", bufs=3) as ipool,
            tc.tile_pool(name="embp", bufs=3, space="PSUM") as tr_ps,
        ):
            for t in range(TT):
                idx_sb = ipool.tile([128, 1], i32)
                nc.sync.dma_start(idx_sb[:], tokidx[t])
                g_sb = gpool.tile([128, DIM], f32, tag="g")
                nc.gpsimd.indirect_dma_start(
                    out=g_sb[:],
                    out_offset=None,
                    in_=tokemb[:],
                    in_offset=bass.IndirectOffsetOnAxis(ap=idx_sb[:, :1], axis=0),
                )
                p_sb = gpool.tile([128, DIM], f32, tag="p")
                nc.sync.dma_start(p_sb[:], possl[t * 128 : (t + 1) * 128, :])
                nc.vector.tensor_add(g_sb[:], g_sb[:], p_sb[:])
                for c in range(DC):
                    tp = tr_ps.tile([128, 128], f32)
                    nc.tensor.transpose(tp[:], g_sb[:, c * 128 : (c + 1) * 128], ident[:])
                    nc.vector.tensor_copy(x_fm[:, c, t * 128 : (t + 1) * 128], tp[:])

        # ---------- helpers ----------
        def rmsnorm_to(dst_bf, ln_pool, ln_ps):
            """dst[:, dc, :] = x_fm * invrms (weights folded into W), fp16."""
            xsq = ln_pool.tile([128, DC, NTOK], bf16, tag="xsq")
            for c in range(DC):
                nc.vector.tensor_mul(xsq[:, c, :], x_fm[:, c, :], x_fm[:, c, :])
            for half in range(2):
                cols = slice(half * 512, half * 512 + 512)
                ssq = ln_ps.tile([1, 512], f32, tag="ssq")
                for c in range(DC):
                    nc.tensor.matmul(
                        ssq[:], ones_bf[:, 0:1], xsq[:, c, cols],
                        start=(c == 0), stop=(c == DC - 1),
                    )
                inv = ln_pool.tile([128, 512], bf16, tag="inv")
                rtmp = ln_pool.tile([128, 512], f32, tag="rtmp")
                nc.scalar.activation(
                    rtmp[0:1, :], ssq[:], AF.Sqrt, bias=eps_sb[0:1], scale=1.0 / DIM
                )
                with nc.allow_low_precision(reason="fp16 invrms feeds fp16 matmul"):
                    nc.vector.reciprocal(inv[0:1, :], rtmp[0:1, :])
                bc = ln_ps.tile([128, 512], f32, tag="bc")
                nc.tensor.matmul(bc[:], ones_bf[0:1, :], inv[0:1, :], start=True, stop=True)
                for c in range(DC):
                    nc.vector.tensor_mul(dst_bf[:, c, cols], x_fm[:, c, cols], bc[:])

        # ---------- layers ----------
        for d in range(ndepth):
            with (
                tc.tile_pool(name=f"ln{d}", bufs=2) as ln_pool,
                tc.tile_pool(name=f"lnp{d}", bufs=2, space="PSUM") as ln_ps,
            ):
                rmsnorm_to(xn_bf, ln_pool, ln_ps)

            # -- qk projection (feature-major) --
            with (
                tc.tile_pool(name=f"wqk{d}", bufs=1) as wpool,
                tc.tile_pool(name=f"qkp{d}", bufs=4, space="PSUM") as qk_ps,
            ):
                w_sb = wpool.tile([128, DC, 1024], bf16)
                nc.sync.dma_start(
                    w_sb[:], wqk[d].rearrange("(dc p) f -> p dc f", p=128)
                )
                for fc in range(8):
                    for half in range(2):
                        cols = slice(half * 512, half * 512 + 512)
                        ps = qk_ps.tile([128, 512], f32)
                        for c in range(DC):
                            nc.tensor.matmul(
                                ps[:],
                                w_sb[:, c, fc * 128 : (fc + 1) * 128],
                                xn_bf[:, c, cols],
                                start=(c == 0), stop=(c == DC - 1),
                            )
                        nc.scalar.copy(qk_bf[:, fc, cols], ps[:])

            # -- rope (feature-major, rotation via PE) --
            with (
                tc.tile_pool(name=f"rp{d}", bufs=4) as rpool,
                tc.tile_pool(name=f"rpp{d}", bufs=3, space="PSUM") as r_ps,
            ):
                for fc in range(8):
                    for half in range(2):
                        cols = slice(half * 512, half * 512 + 512)
                        rot = r_ps.tile([128, 512], f32)
                        nc.tensor.matmul(
                            rot[:], rmat_sb[:], qk_bf[:, fc, cols], start=True, stop=True
                        )
                        rot_sb = rpool.tile([128, 512], bf16, tag="rot")
                        nc.vector.tensor_copy(rot_sb[:], rot[:])
                        t1 = rpool.tile([128, 512], bf16, tag="t1")
                        nc.vector.tensor_mul(t1[:], qk_bf[:, fc, cols], cos_sb[:, cols])
                        nc.vector.tensor_mul(rot_sb[:], rot_sb[:], sin_sb[:, cols])
                        nc.vector.tensor_add(qkr_bf[:, fc, cols], t1[:], rot_sb[:])

            # -- v + mix projection (token-major / key-major) --
            v_tm = v0_tm if d == 0 else v1_tm
            with (
                tc.tile_pool(name=f"wv{d}", bufs=1) as wvpool,
                tc.tile_pool(name=f"vp{d}", bufs=3, space="PSUM") as v_ps,
                tc.tile_pool(name=f"vm{d}", bufs=3) as vtmp,
            ):
                wv_sb = wvpool.tile([128, DC, 520], bf16)
                nc.sync.dma_start(
                    wv_sb[:], wvm[d].rearrange("(dc p) f -> p dc f", p=128)
                )
                for t in range(TT):
                    trow = slice(t * 128, (t + 1) * 128)
                    ps = v_ps.tile([128, 512], f32, tag="v")
                    for c in range(DC):
                        nc.tensor.matmul(
                            ps[:], xn_bf[:, c, trow], wv_sb[:, c, 0:512],
                            start=(c == 0), stop=(c == DC - 1),
                        )
                    if d == 0:
                        nc.scalar.copy(
                            v_tm[:, t, :, 0:DH],
                            ps[:].rearrange("p (h e) -> p h e", h=HEADS),
                        )
                    else:
                        ps8 = v_ps.tile([128, 8], f32, tag="m")
                        for c in range(DC):
                            nc.tensor.matmul(
                                ps8[:], xn_bf[:, c, trow], wv_sb[:, c, 512:520],
                                start=(c == 0), stop=(c == DC - 1),
                            )
                        mixf = vtmp.tile([128, 8], f32, tag="mixf")
                        nc.vector.tensor_add(mixf[:], ps8[:], vb_sb[:])
                        mix = vtmp.tile([128, 8], bf16, tag="mix")
                        nc.scalar.activation(mix[:], mixf[:], AF.Sigmoid)
                        v1 = vtmp.tile([128, HEADS, DH], bf16, tag="v1")
                        nc.scalar.copy(v1[:], ps[:].rearrange("p (h e) -> p h e", h=HEADS))
                        dv = vtmp.tile([128, HEADS, DH], bf16, tag="dv")
                        nc.vector.tensor_sub(dv[:], v0_tm[:, t, :, 0:DH], v1[:])
                        nc.vector.tensor_mul(
                            dv[:], dv[:], mix[:, :, None].to_broadcast([128, HEADS, DH])
                        )
                        nc.vector.tensor_add(v_tm[:, t, :, 0:DH], v1[:], dv[:])
                nc.vector.memset(v_tm[:, :, :, DH : DH + 1], 1.0)

            # -- attention --
            with (
                tc.tile_pool(name=f"pm{d}", bufs=1) as pmpool,
                tc.tile_pool(name=f"ep{d}", bufs=6) as epool,
                tc.tile_pool(name=f"at{d}", bufs=4) as apool,
                tc.tile_pool(name=f"simp{d}", bufs=2, space="PSUM") as sim_ps,
                tc.tile_pool(name=f"op{d}", bufs=2, space="PSUM") as o_ps_pool,
                tc.tile_pool(name=f"bcp{d}", bufs=2, space="PSUM") as bc_ps_pool,
            ):
                pmk_sb = pmpool.tile([128, 4, PM], bf16)
                nc.sync.dma_start(pmk_sb[:], pmk[d])
                pmv_sb = pmpool.tile([PM, HEADS, DH + 1], bf16)
                nc.sync.dma_start(pmv_sb[:], pmv[d])
                for s in range(NSEG):
                    scols = slice(s * 512, (s + 1) * 512)
                    for h in range(HEADS):
                        base = (h % 2) * 64
                        fcq = h // 2
                        fck = 4 + h // 2
                        q_ap = qkr_bf[base : base + 64, fcq, scols]
                        e_tiles = []
                        for c in range(4):
                            sp = sim_ps.tile([128, 512], f32, tag="sim")
                            k_ap = qkr_bf[
                                base : base + 64, fck,
                                s * 512 + c * 128 : s * 512 + (c + 1) * 128,
                            ]
                            nc.tensor.matmul(sp[:], k_ap, q_ap, start=True, stop=True)
                            e_c = epool.tile([128, 512], bf16, tag="e")
                            nc.scalar.activation(
                                e_c[:, c * 128 :], sp[:, c * 128 :], AF.Exp,
                                scale=DH**-0.5,
                            )
                            if c > 0:
                                nc.gpsimd.memset(e_c[:, : c * 128], 0.0)
                            nc.vector.tensor_mul(
                                e_c[:, c * 128 : (c + 1) * 128],
                                e_c[:, c * 128 : (c + 1) * 128],
                                tri_sb[:],
                            )
                            e_tiles.append(e_c)
                        pp = sim_ps.tile([PM, 512], f32, tag="sim")
                        nc.tensor.matmul(
                            pp[:], pmk_sb[base : base + 64, fcq, :], q_ap,
                            start=True, stop=True,
                        )
                        e_pm = epool.tile([PM, 512], bf16, tag="epm")
                        nc.scalar.activation(e_pm[:], pp[:], AF.Exp, scale=DH**-0.5)
                        # o (rows 0..63) + denom (row 64)
                        op = o_ps_pool.tile([DH + 1, 512], f32)
                        for c in range(4):
                            nc.tensor.matmul(
                                op[:],
                                v_tm[:, 4 * s + c, h, :],
                                e_tiles[c][:],
                                start=(c == 0), stop=False,
                            )
                        nc.tensor.matmul(
                            op[:], pmv_sb[:, h, :], e_pm[:], start=False, stop=True
                        )
                        invd = apool.tile([128, 512], bf16, tag="invd")
                        with nc.allow_low_precision(reason="fp16 inv-denom"):
                            nc.vector.reciprocal(invd[64:65, :], op[64:65, :])
                        bc = bc_ps_pool.tile([64, 512], f32)
                        nc.tensor.matmul(
                            bc[:], ones_bf[64:65, 0:64], invd[64:65, :],
                            start=True, stop=True,
                        )
                        o_f = apool.tile([64, 512], f32, tag="of")
                        nc.scalar.copy(o_f[:], op[0:64, :])
                        if h % 2 == 0:
                            nc.vector.tensor_mul(o_asm[0:64, fcq, scols], o_f[:], bc[:])
                        else:
                            o_tmp = apool.tile([64, 512], bf16, tag="otmp")
                            nc.vector.tensor_mul(o_tmp[:], o_f[:], bc[:])
                            nc.sync.dma_start(o_asm[64:128, fcq, scols], o_tmp[:])

            # -- output projection + residual --
            with (
                tc.tile_pool(name=f"wo{d}", bufs=1) as wopool,
                tc.tile_pool(name=f"wop{d}", bufs=3, space="PSUM") as wo_ps,
            ):
                wo_sb = wopool.tile([128, 4, 512], bf16)
                nc.sync.dma_start(
                    wo_sb[:], woutw[d].rearrange("(kc p) m -> p kc m", p=128)
                )
                for mc in range(DC):
                    for half in range(2):
                        cols = slice(half * 512, half * 512 + 512)
                        ps = wo_ps.tile([128, 512], f32)
                        for kc in range(4):
                            nc.tensor.matmul(
                                ps[:],
                                wo_sb[:, kc, mc * 128 : (mc + 1) * 128],
                                o_asm[:, kc, cols],
                                start=(kc == 0), stop=(kc == 3),
                            )
                        nc.vector.tensor_add(x_fm[:, mc, cols], x_fm[:, mc, cols], ps[:])

            # -- GEGLU FF --
            with (
                tc.tile_pool(name=f"ln2{d}", bufs=2) as ln_pool,
                tc.tile_pool(name=f"ln2p{d}", bufs=2, space="PSUM") as ln_ps,
            ):
                rmsnorm_to(xn_bf, ln_pool, ln_ps)
            with (
                tc.tile_pool(name=f"wf{d}", bufs=1) as wfpool,
                tc.tile_pool(name=f"fb{d}", bufs=1) as fbpool,
                tc.tile_pool(name=f"ffp{d}", bufs=2, space="PSUM") as ff_ps,
                tc.tile_pool(name=f"fft{d}", bufs=4) as ftmp,
            ):
                w1_sb = wfpool.tile([128, DC, 2 * FFI], bf16)
                nc.sync.dma_start(
                    w1_sb[:], w1[d].rearrange("(dc p) f -> p dc f", p=128)
                )
                w2_sb = wfpool.tile([128, 11, 512], bf16)
                nc.sync.dma_start(
                    w2_sb[:, 0:10, :],
                    w2[d, 0:1280, :].rearrange("(kb p) m -> p kb m", p=128),
                )
                nc.sync.dma_start(w2_sb[0:85, 10, :], w2[d, 1280:1365, :])
                b1a = []
                b1g = []
                for i in range(11):
                    pa = FB[i][1]
                    ta = fbpool.tile([128, 1], f32, name=f"b1a{d}_{i}")
                    tg = fbpool.tile([128, 1], f32, name=f"b1g{d}_{i}")
                    nc.sync.dma_start(ta[0:pa, :], b1[d, 256 * i : 256 * i + pa, :])
                    nc.sync.dma_start(
                        tg[0:pa, :], b1[d, 256 * i + pa : 256 * i + 2 * pa, :]
                    )
                    b1a.append(ta)
                    b1g.append(tg)
                b2_sb = fbpool.tile([128, DC], f32)
                nc.sync.dma_start(b2_sb[:], b2[d])
                for i in range(11):
                    pa = FB[i][1]
                    for half in range(2):
                        cols = slice(half * 512, half * 512 + 512)
                        aps = ff_ps.tile([128, 512], f32, tag="a")
                        gps = ff_ps.tile([128, 512], f32, tag="g")
                        for c in range(DC):
                            nc.tensor.matmul(
                                aps[0:pa, :],
                                w1_sb[:, c, 256 * i : 256 * i + pa],
                                xn_bf[:, c, cols],
                                start=(c == 0), stop=(c == DC - 1),
                            )
                        for c in range(DC):
                            nc.tensor.matmul(
                                gps[0:pa, :],
                                w1_sb[:, c, 256 * i + pa : 256 * i + 2 * pa],
                                xn_bf[:, c, cols],
                                start=(c == 0), stop=(c == DC - 1),
                            )
                        g_sb = ftmp.tile([128, 512], bf16, tag="gs")
                        nc.scalar.activation(
                            g_sb[0:pa, :], gps[0:pa, :], AF.Identity, bias=b1g[i][0:pa]
                        )
                        sig_sb = ftmp.tile([128, 512], bf16, tag="sg")
                        nc.scalar.activation(
                            sig_sb[0:pa, :], gps[0:pa, :], AF.Sigmoid, bias=b1g[i][0:pa]
                        )
                        a_sb = ftmp.tile([128, 512], bf16, tag="as")
                        nc.vector.tensor_scalar_add(
                            a_sb[0:pa, :], aps[0:pa, :], b1a[i][0:pa]
                        )
                        nc.vector.tensor_mul(
                            a_sb[0:pa, :], a_sb[0:pa, :], g_sb[0:pa, :]
                        )
                        nc.vector.tensor_mul(
                            h_sb[0:pa, i, cols], a_sb[0:pa, :], sig_sb[0:pa, :]
                        )
                for mc in range(DC):
                    for half in range(2):
                        cols = slice(half * 512, half * 512 + 512)
                        ps = ff_ps.tile([128, 512], f32, tag="o2")
                        for kb in range(11):
                            pa = FB[kb][1]
                            nc.tensor.matmul(
                                ps[:],
                                w2_sb[0:pa, kb, mc * 128 : (mc + 1) * 128],
                                h_sb[0:pa, kb, cols],
                                start=(kb == 0), stop=(kb == 10),
                            )
                        t_sb = ftmp.tile([128, 512], f32, tag="t2")
                        nc.scalar.activation(
                            t_sb[:], ps[:], AF.Identity, bias=b2_sb[:, mc : mc + 1]
                        )
                        nc.vector.tensor_add(x_fm[:, mc, cols], x_fm[:, mc, cols], t_sb[:])

        # ---------- final norm + logits ----------
        with (
            tc.tile_pool(name="lnf", bufs=2) as ln_pool,
            tc.tile_pool(name="lnfp", bufs=2, space="PSUM") as ln_ps,
        ):
            rmsnorm_to(xn_bf, ln_pool, ln_ps)
        with (
            tc.tile_pool(name="wl", bufs=3) as wlpool,
            tc.tile_pool(name="lg", bufs=4, space="PSUM") as lg_ps,
            tc.tile_pool(name="lo", bufs=6) as lopool,
        ):
            for vc in range(nvc):
                vcols = slice(vc * VCH, (vc + 1) * VCH)
                wl_sb = wlpool.tile([128, DC, VCH], bf16)
                nc.sync.dma_start(
                    wl_sb[:], wl[vc].rearrange("p (dc v) -> p dc v", dc=DC)
                )
                for t in range(TT):
                    trow = slice(t * 128, (t + 1) * 128)
                    ps = lg_ps.tile([128, VCH], f32)
                    for c in range(DC):
                        nc.tensor.matmul(
                            ps[:], xn_bf[:, c, trow], wl_sb[:, c, :],
                            start=(c == 0), stop=(c == DC - 1),
                        )
                    o_sb = lopool.tile([128, VCH], f32)
                    if (vc * TT + t) % 2 == 0:
                        nc.scalar.copy(o_sb[:], ps[:])
                    else:
                        nc.vector.tensor_copy(o_sb[:], ps[:])
                    nc.sync.dma_start(out[trow, vcols], o_sb[:])

        persist.release()
        const.release()

    nc.compile()
    return nc


def _host_prep(inputs):
    """Build the shared (weights) and per-core input maps."""
    bf = np.float16
    f = lambda x: np.ascontiguousarray(np.asarray(x, np.float32))
    tokens = np.asarray(inputs["tokens"]).astype(np.int32)
    tok_emb = f(inputs["tok_emb"])
    pos_emb = f(inputs["pos_emb"])
    anw = f(inputs["attn_norm_w"])  # [2,512]
    Wqkv = f(inputs["Wqkv"])  # [2,512,1536]
    persist_mem = f(inputs["persist_mem"])  # [2,2,8,4,64]
    Wout = f(inputs["Wout"])
    vmix_w = f(inputs["vmix_w"])  # [2,512,8]
    vmix_b = f(inputs["vmix_b"])  # [2,8]
    fnw = f(inputs["ff_norm_w"])
    ff_w1 = f(inputs["ff_w1"])  # [2,512,2730]
    ff_b1 = f(inputs["ff_b1"])  # [2,2730]
    ff_w2 = f(inputs["ff_w2"])  # [2,1365,512]
    ff_b2 = f(inputs["ff_b2"])  # [2,512]
    finw = f(inputs["final_norm_w"])  # [512]
    w_logits = f(inputs["w_logits"])  # [512,32000]

    # ---- shared tensors ----
    wqk = np.ascontiguousarray((anw[:, :, None] * Wqkv[:, :, :1024]).astype(bf))
    wvm = np.ascontiguousarray(
        np.concatenate(
            [anw[:, :, None] * Wqkv[:, :, 1024:], anw[:, :, None] * vmix_w], axis=2
        ).astype(bf)
    )
    vmixb = np.broadcast_to(vmix_b[1], (128, HEADS)).astype(np.float32).copy()
    # pmk lhsT: [d, r(128), pair(4), pm] ; r<64 -> head 2*pair dh=r ; r>=64 -> head 2*pair+1
    pmk = np.zeros((DEPTH, 128, 4, PM), np.float32)
    for pair in range(4):
        pmk[:, 0:64, pair, :] = persist_mem[:, 0, 2 * pair].transpose(0, 2, 1)
        pmk[:, 64:128, pair, :] = persist_mem[:, 0, 2 * pair + 1].transpose(0, 2, 1)
    pmk = pmk.astype(bf)
    pmv = np.ones((DEPTH, PM, HEADS, DH + 1), np.float32)
    pmv[:, :, :, 0:DH] = persist_mem[:, 1].transpose(0, 2, 1, 3)
    pmv = pmv.astype(bf)
    woutw = Wout.astype(bf)
    # w1: interleave a/g blocks of 128 (last 85), fold ff norm weight
    w1s = fnw[:, :, None] * ff_w1
    w1 = np.zeros((DEPTH, DIM, 2 * FFI), np.float32)
    b1 = np.zeros((DEPTH, 2 * FFI, 1), np.float32)
    for i, (off, pa) in enumerate(FB):
        w1[:, :, 256 * i : 256 * i + pa] = w1s[:, :, off : off + pa]
        w1[:, :, 256 * i + pa : 256 * i + 2 * pa] = w1s[:, :, FFI + off : FFI + off + pa]
        b1[:, 256 * i : 256 * i + pa, 0] = ff_b1[:, off : off + pa]
        b1[:, 256 * i + pa : 256 * i + 2 * pa, 0] = ff_b1[:, FFI + off : FFI + off + pa]
    w1 = w1.astype(bf)
    w2 = ff_w2.astype(bf)
    b2 = np.ascontiguousarray(
        ff_b2.reshape(DEPTH, DC, 128).transpose(0, 2, 1)
    ).astype(np.float32)
    wl_eff = (finw[:, None] * w_logits).astype(bf)  # [512, 32000]
    # swizzle to [vc, p, dc*VCH+j] so per-partition DMA lines are contiguous
    wl = np.ascontiguousarray(
        wl_eff.reshape(DC, 128, NVC, VCH).transpose(2, 1, 0, 3).reshape(NVC, 128, DC * VCH)
    )
    # rope rotation matrix (lhsT): rot = x[2i] -> out[2i+1], -x[2i+1] -> out[2i]
    rmat = np.zeros((128, 128), np.float32)
    for blk in range(2):
        o = blk * 64
        for i in range(32):
            rmat[o + 2 * i + 1, o + 2 * i] = -1.0
            rmat[o + 2 * i, o + 2 * i + 1] = 1.0
    rmat = rmat.astype(bf)
    tri = np.triu(np.ones((128, 128), np.float32)).astype(bf)  # allowed q>=k

    flat_tokens = tokens.reshape(-1)
    inv = (10000.0 ** (-np.arange(0, DH, 2, dtype=np.float64) / DH)).astype(np.float64)
    row_pair = (np.arange(128) % 64) // 2  # pair index per fm row

    in_maps = []
    for c in range(NCORES):
        pos = (c * NTOK + np.arange(NTOK)) % S
        fr = pos[:, None].astype(np.float64) * inv[None, :]  # [1024, 32]
        cosb = np.cos(fr)[:, row_pair].T.astype(bf)  # [128, 1024]
        sinb = np.sin(fr)[:, row_pair].T.astype(bf)
        in_maps.append(
            {
                "tokidx": np.ascontiguousarray(
                    flat_tokens[c * NTOK : (c + 1) * NTOK].reshape(TT, 128, 1)
                ),
                "possl": np.ascontiguousarray(pos_emb[pos]),
                "tokemb": tok_emb,
                "cosb": np.ascontiguousarray(cosb),
                "sinb": np.ascontiguousarray(sinb),
                "rmat": rmat,
                "trimask": tri,
                "wqk": wqk,
                "wvm": wvm,
                "vmixb": vmixb,
                "pmk": pmk,
                "pmv": pmv,
                "woutw": woutw,
                "w1": w1,
                "b1": b1,
                "w2": w2,
                "b2": b2,
                "wl": wl,
            }
        )
    return in_maps


def kernel(**inputs):
    from concourse.bass_utils import run_bass_kernel_spmd

    if "nc" not in _cache:
        _cache["nc"] = _build_program()
    nc = _cache["nc"]
    in_maps = _host_prep(inputs)
    res = run_bass_kernel_spmd(nc, in_maps, core_ids=list(range(NCORES)))
    outs = [res.results[c]["out"] for c in range(NCORES)]
    return np.concatenate(outs, axis=0).reshape(B, S, VOCAB).astype(np.float32)
